# revision 1
# baseline (speedup 1.0000x reference)
"""Trainium2 Bass kernel for nn_Attention_43198781063919.

Computes, for inputs sent1/sent2 [32, 512, 1024] f32 and W [6, 1024, 1024] f32:
    scores[b,o] = sent1[b] @ W[o] @ sent2[b].T          (512 x 512)
    out[b,o]    = top-10 values of scores[b,o]          ([32, 6, 10] f32)

Strategy (8 NeuronCores, data-parallel over batch):
  - Each core handles 4 batches x 6 W matrices = 24 score matrices.
  - Host-side sharding casts operands to fp16 (11-bit mantissa, ~4e-4 top-10
    rel err) and pre-transposes sent1/sent2 to [H, L] so the PE contraction
    dim lands on SBUF partitions with plain contiguous DMA loads.
  - Stage 1: A.T[q,i] = (sent1[b] @ W[o]).T accumulated over 8 p-chunks in
    PSUM, copied to SBUF as fp16 by ScalarE.
  - Stage 2: scores[i,j] accumulated over 8 q-chunks; VectorE max8 reads each
    PSUM tile directly -> per-partition top-8 candidates.
  - Top-10: global top-10 is contained in the per-partition top-8 candidates
    (the only failure mode is >8 of the global top-10 landing in one
    partition's 4 score rows; probability ~1e-16 for random scores, and the
    result is verified exact against the reference on the actual inputs).
    Candidates reduce 32->8 per partition, flatten to 4 SBUF quarter-rows
    per (b,o), then two exact max8/match_replace8/max8 rounds (256-wide,
    then 64-wide) produce the sorted top-16, DMA'd straight to DRAM; the
    host keeps the first 10 of each row.
"""
import numpy as np
from contextlib import ExitStack

import concourse.bass as bass  # noqa: F401
from concourse import bacc
import concourse.tile as tile
from concourse import mybir
from concourse import bass_utils

dt = mybir.dt

B, L, H, OUT_DIM, TOPK = 32, 512, 1024, 6, 10
NCORES = 8
BPC = B // NCORES          # batches per core
NR = BPC * OUT_DIM         # score matrices per core
PCH = H // 128             # 8 contraction chunks

_NC = None


def _build():
    nc = bacc.Bacc("TRN2", debug=False, num_devices=NCORES)
    s1T = nc.dram_tensor("s1T", [BPC, H, L], dt.float16, kind="ExternalInput").ap()
    s2T = nc.dram_tensor("s2T", [BPC, H, L], dt.float16, kind="ExternalInput").ap()
    W = nc.dram_tensor("W", [OUT_DIM, H, H], dt.float16, kind="ExternalInput").ap()
    out = nc.dram_tensor("out", [NR, 16], dt.float32, kind="ExternalOutput").ap()

    with tile.TileContext(nc) as tc:
        with ExitStack() as ctx:
            sentp = ctx.enter_context(tc.tile_pool(name="sent", bufs=2))
            wpool = ctx.enter_context(tc.tile_pool(name="w", bufs=2))
            atp = ctx.enter_context(tc.tile_pool(name="at", bufs=2))
            candp = ctx.enter_context(tc.tile_pool(name="cand", bufs=3))
            cpool = ctx.enter_context(tc.tile_pool(name="c", bufs=1))
            pa = ctx.enter_context(tc.tile_pool(name="pa", bufs=3, space="PSUM"))
            ps = ctx.enter_context(tc.tile_pool(name="ps", bufs=4, space="PSUM"))

            C = cpool.tile([4 * NR, 256], dt.float32)

            # PE warmup: junk matmuls on a zeroed tile keep the HAM activity
            # window busy while the first input DMAs land, so the real matmul
            # stream starts at the warm 2.4 GHz clock.
            warm_src = candp.tile([128, 640], dt.float16, tag="warm_src")
            nc.vector.memset(warm_src[:], 0.0)
            warm_ps = ctx.enter_context(tc.tile_pool(name="warm", bufs=1, space="PSUM"))
            wps = warm_ps.tile([128, 512], dt.float32)
            for _ in range(14):
                nc.tensor.matmul(wps[:], warm_src[:, 0:128], warm_src[:, 128:640],
                                 start=True, stop=True)

            for b in range(BPC):
                s1t = sentp.tile([128, PCH * L], dt.float16, tag="s1t")
                s2t = sentp.tile([128, PCH * L], dt.float16, tag="s2t")
                for o in range(OUT_DIM):
                    wt = wpool.tile([128, PCH * H], dt.float16, tag="wt")
                    # W[o] in four column quarters and sent halves, interleaved
                    # so the first stage-1 accumulation group is gated on only
                    # ~1MB (first W quarter + first s1t half)
                    wt4 = wt[:].rearrange("p (k q) -> p k q", k=PCH)
                    Wo4 = W[o].rearrange("(k p) q -> p k q", p=128)
                    if b == 0 and o == 0:
                        # finest interleave for the very first gate: the first
                        # accumulation group starts after ~0.5MB has landed
                        s1v = s1t[:].rearrange("p (k i) -> p k i", k=PCH)
                        s1d = s1T[b].rearrange("(k p) i -> p k i", p=128)
                        E = H // 8
                        nc.sync.dma_start(wt4[:, :, 0:E], Wo4[:, :, 0:E])
                        nc.sync.dma_start(s1v[:, 0:2, :], s1d[:, 0:2, :])
                        nc.sync.dma_start(s1v[:, 2:4, :], s1d[:, 2:4, :])
                        nc.sync.dma_start(s1v[:, 4:6, :], s1d[:, 4:6, :])
                        nc.sync.dma_start(wt4[:, :, E:2 * E], Wo4[:, :, E:2 * E])
                        nc.sync.dma_start(s1v[:, 6:8, :], s1d[:, 6:8, :])
                        for e in range(2, 8):
                            nc.sync.dma_start(wt4[:, :, e * E:(e + 1) * E],
                                              Wo4[:, :, e * E:(e + 1) * E])
                    else:
                        Q = H // 4
                        nc.sync.dma_start(wt4[:, :, 0:Q], Wo4[:, :, 0:Q])
                        if o == 0:
                            s1v = s1t[:].rearrange("p (k i) -> p k i", k=PCH)
                            s1d = s1T[b].rearrange("(k p) i -> p k i", p=128)
                            nc.sync.dma_start(s1v[:, 0:4, :], s1d[:, 0:4, :])
                            nc.sync.dma_start(wt4[:, :, Q:2 * Q], Wo4[:, :, Q:2 * Q])
                            nc.sync.dma_start(s1v[:, 4:8, :], s1d[:, 4:8, :])
                        else:
                            nc.sync.dma_start(wt4[:, :, Q:2 * Q], Wo4[:, :, Q:2 * Q])
                        nc.sync.dma_start(wt4[:, :, 2 * Q:3 * Q], Wo4[:, :, 2 * Q:3 * Q])
                        nc.sync.dma_start(wt4[:, :, 3 * Q:4 * Q], Wo4[:, :, 3 * Q:4 * Q])
                    if o == 0:
                        nc.sync.dma_start(
                            s2t[:].rearrange("p (k j) -> p k j", k=PCH),
                            s2T[b].rearrange("(k p) j -> p k j", p=128),
                        )
                    # stage 1: A.T[qc*128:(qc+1)*128, :] = (s1[b] @ W[o]).T chunk
                    at_sb = atp.tile([128, PCH * L], dt.float16, tag="at")
                    for qc in range(PCH):
                        acc = pa.tile([128, L], dt.float32, tag="pa")
                        for pc in range(PCH):
                            nc.tensor.matmul(
                                acc[:],
                                wt[:, pc * H + qc * 128:pc * H + qc * 128 + 128],
                                s1t[:, pc * L:(pc + 1) * L],
                                start=(pc == 0), stop=(pc == PCH - 1),
                            )
                        nc.scalar.copy(at_sb[:, qc * L:(qc + 1) * L], acc[:])
                    # stage 2: scores i-chunks; top-8 per partition from PSUM
                    cand = candp.tile([128, 40], dt.float32, tag="cand")
                    for ic in range(4):
                        sc = ps.tile([128, L], dt.float32, tag="ps")
                        for qc in range(PCH):
                            nc.tensor.matmul(
                                sc[:],
                                at_sb[:, qc * L + ic * 128:qc * L + ic * 128 + 128],
                                s2t[:, qc * L:(qc + 1) * L],
                                start=(qc == 0), stop=(qc == PCH - 1),
                            )
                        nc.vector.max(cand[:, ic * 8:(ic + 1) * 8], sc[:])
                    # reduce 32 -> 8 per partition before the flatten so the
                    # final cross-partition top-k runs on 256-wide quarter rows
                    nc.vector.max(cand[:, 32:40], cand[:, 0:32])
                    r = b * OUT_DIM + o
                    # quarter-row flatten: cand partitions 32a..32a+31 land on
                    # C partition 4r+a, 256 candidates each (source stays a
                    # plain partition-major AP; only the dest is rearranged)
                    nc.sync.dma_start(
                        C[4 * r:4 * r + 4, :].rearrange("a (p f) -> a p f", p=32),
                        cand[:, 32:40],
                    )

            # level 2a: exact sorted top-16 of each 256-wide quarter row
            q16 = candp.tile([4 * NR, 16], dt.float32, tag="q16")
            nc.vector.max(q16[:, 0:8], C[:])
            replq = cpool.tile([4 * NR, 256], dt.float32)
            nc.vector.match_replace(replq[:], q16[:, 0:8], C[:], -3.0e38)
            nc.vector.max(q16[:, 8:16], replq[:])
            # merge quarters: one 64-wide row per (b,o)
            C2 = candp.tile([NR, 64], dt.float32, tag="c2")
            nc.sync.dma_start(
                C2[:].rearrange("r (p f) -> r p f", p=4),
                q16[:],
            )
            # level 2b: exact sorted top-16 of each 64-wide merged row
            t8 = candp.tile([NR, 8], dt.float32, tag="t8")
            nc.vector.max(t8[:], C2[:])
            repl = candp.tile([NR, 64], dt.float32, tag="repl")
            nc.vector.match_replace(repl[:], t8[:], C2[:], -3.0e38)
            n8 = candp.tile([NR, 8], dt.float32, tag="n8")
            nc.sync.dma_start(out[:, 0:8], t8[:])
            nc.vector.max(n8[:], repl[:])
            nc.sync.dma_start(out[:, 8:16], n8[:])

    nc.compile()
    return nc


def _in_maps(sent1, sent2, W):
    maps = []
    Wh = np.ascontiguousarray(W).astype(np.float16)
    for c in range(NCORES):
        sl = slice(c * BPC, (c + 1) * BPC)
        maps.append({
            "s1T": np.ascontiguousarray(np.asarray(sent1)[sl].transpose(0, 2, 1)).astype(np.float16),
            "s2T": np.ascontiguousarray(np.asarray(sent2)[sl].transpose(0, 2, 1)).astype(np.float16),
            "W": Wh,
        })
    return maps


def _gather(results):
    outs = []
    for c in range(NCORES):
        o = results[c]["out"]                      # [24, 16]
        outs.append(o[:, :TOPK].reshape(BPC, OUT_DIM, TOPK))
    return np.concatenate(outs, axis=0).astype(np.float32)


def kernel(sent1, sent2, W):
    global _NC
    if _NC is None:
        _NC = _build()
    res = bass_utils.run_bass_kernel_spmd(
        _NC, _in_maps(sent1, sent2, W), core_ids=list(range(NCORES))
    )
    return _gather(res.results)


def run_traced(sent1, sent2, W):
    """Like kernel() but with NTFF tracing; returns (output, exec_time_ns).

    The caller must install the antenv.axon_hooks NTFF profile hook first
    (see test.py); without it exec_time_ns is None.
    """
    global _NC
    if _NC is None:
        _NC = _build()
    res = bass_utils.run_bass_kernel_spmd(
        _NC, _in_maps(sent1, sent2, W), core_ids=list(range(NCORES)), trace=True
    )
    return _gather(res.results), res.exec_time_ns, res



# revision 22
# speedup vs baseline: 1.3714x; 1.3714x over previous
"""Trainium2 Bass kernel for nn_Attention_43198781063919.

Computes, for inputs sent1/sent2 [32, 512, 1024] f32 and W [6, 1024, 1024] f32:
    scores[b,o] = sent1[b] @ W[o] @ sent2[b].T          (512 x 512)
    out[b,o]    = top-10 values of scores[b,o]          ([32, 6, 10] f32)

Strategy (8 NeuronCores, data-parallel over batch; fp8 DoubleRow + exact
rescore):
  Phase 1 (fp8 e4m3, PE DoubleRow = 2 MACs/cell/cycle):
  - A.T[q,i] = (s1[b] @ 256*W[o]).T accumulated over 4 DoubleRow p-pairs in
    PSUM; ScalarE writes it back as fp8 with scale 1/64.  The o-outer /
    b-inner loop reuses each stationary W chunk for 4 batches.
  - scores8 = A.T.T @ s2.T per i-chunk; VectorE max8 gives per-row maxima.
  - Per (b,o): the 512 row-maxima are flattened to one partition, and two
    max/max_index/match_replace rounds give the top-16 candidate row ids
    (rows that can contain the true top-10; fp8 error ~1 abs vs a ~5+ rank
    margin, miss probability ~1e-8).  The ids are wrapped into the 16-
    partition layout dma_gather wants via partition_broadcast + a masked
    reduce.
  Phase 2 (exact fp16 rescore of 16 rows x 512 cols per (b,o)):
  - dma_gather (transpose) pulls the candidate s1 rows from HBM in
    [p, pc, cand] layout (host pre-permutes rows so flatten positions are
    row ids).
  - V = W[o].T-chunks @ gathered -> exact A rows; then per (b,o) col-packed
    (tile_position) matmuls against s2.T give exact candidate scores.
  - max8 + match_replace top-16 per (b,o), DMA'd to DRAM; host keeps 10.
    Output values come only from the exact fp16 path (no descaling needed).
"""
import numpy as np
from contextlib import ExitStack

import concourse.bass as bass  # noqa: F401
from concourse import bacc
import concourse.tile as tile
from concourse import mybir
from concourse import bass_utils

dt = mybir.dt
DR = mybir.MatmulPerfMode.DoubleRow

B, L, H, OUT_DIM, TOPK = 32, 512, 1024, 6, 10
NCORES = 8
BPC = B // NCORES          # batches per core
NR = BPC * OUT_DIM         # score matrices per core
PCH = H // 128             # 8 contraction chunks of 128
QP = PCH // 2              # 4 DoubleRow chunk-pairs
W_SCALE = 256.0            # fp8 range scaling for W (std 0.02 -> 5.1)
A_SCALE = 1.0 / 64.0       # A = s1@(256W) has std ~164 -> fp8 std ~2.6
NCAND = 32                 # rescored candidate rows per (b, o); worst observed
                           # true-top10 row rank in fp8 rowmax order is 23
NCB = OUT_DIM * NCAND      # candidate rows per batch (192)

_NC = None


def _build():
    nc = bacc.Bacc("TRN2", debug=False, num_devices=NCORES)
    s1T8 = nc.dram_tensor("s1T8", [BPC, H, L], dt.float8e4, kind="ExternalInput").ap()
    s2T8 = nc.dram_tensor("s2T8", [BPC, H, L], dt.float8e4, kind="ExternalInput").ap()
    W8 = nc.dram_tensor("W8", [OUT_DIM, H, H], dt.float8e4, kind="ExternalInput").ap()
    s1p = nc.dram_tensor("s1p", [BPC, L, H], dt.float16, kind="ExternalInput").ap()
    s2Tb = nc.dram_tensor("s2Tb", [BPC, H, L], dt.float16, kind="ExternalInput").ap()
    Wb = nc.dram_tensor("Wb", [OUT_DIM, H, H], dt.float16, kind="ExternalInput").ap()
    iotac = nc.dram_tensor("iotac", [128, 4], dt.float32, kind="ExternalInput").ap()
    out = nc.dram_tensor("out", [NR, 16], dt.float32, kind="ExternalOutput").ap()

    with tile.TileContext(nc) as tc:
        with ExitStack() as ctx:
            # SBUF pools
            sentp = ctx.enter_context(tc.tile_pool(name="sent", bufs=1))
            wpool = ctx.enter_context(tc.tile_pool(name="w", bufs=2))
            atp = ctx.enter_context(tc.tile_pool(name="at", bufs=2))
            rmp = ctx.enter_context(tc.tile_pool(name="rm", bufs=2))
            rowp = ctx.enter_context(tc.tile_pool(name="row", bufs=2))
            smp = ctx.enter_context(tc.tile_pool(name="sm", bufs=2))
            gtp = ctx.enter_context(tc.tile_pool(name="gt", bufs=1))
            wbp = ctx.enter_context(tc.tile_pool(name="wb", bufs=2))
            vsp = ctx.enter_context(tc.tile_pool(name="vs", bufs=2))
            cndp = ctx.enter_context(tc.tile_pool(name="cnd", bufs=1))
            # PSUM pools: acc0..3 + fps (pa, bufs=1) = 5 banks,
            # sc (ps, bufs=2) = 2 banks, vps (pv, bufs=1) = 1 bank -> 8
            pa = ctx.enter_context(tc.tile_pool(name="pa", bufs=1, space="PSUM"))
            ps = ctx.enter_context(tc.tile_pool(name="ps", bufs=2, space="PSUM"))
            pv = ctx.enter_context(tc.tile_pool(name="pv", bufs=1, space="PSUM"))

            # persistent fp8 inputs
            s1t8 = sentp.tile([128, BPC, PCH, L], dt.float8e4, tag="s1t8")
            s2t8 = sentp.tile([128, BPC, PCH, L], dt.float8e4, tag="s2t8")
            s2tb = sentp.tile([128, BPC, PCH, L], dt.float16, tag="s2tb")
            iot = sentp.tile([128, 4], dt.float32, tag="iot")
            cnd_all = cndp.tile([128, OUT_DIM * 8], dt.float32, tag="cnda")
            # e0 matrix + id staging rows for the PE partition-broadcast
            onesE = sentp.tile([32, 128], dt.float16, tag="onesE")
            nc.vector.memset(onesE[:], 0.0)
            nc.vector.memset(onesE[0:1, :], 1.0)
            IfAll = sentp.tile([32, NR * NCAND], dt.float16, tag="IfAll")
            nc.vector.memset(IfAll[:], 0.0)

            # PE warmup: junk DoubleRow matmuls on a zeroed tile keep the HAM
            # activity window busy while the first input DMAs land.
            warm_src = smp.tile([128, 1280], dt.float8e4, tag="warm")
            nc.vector.memset(warm_src[:], 0.0)
            wps = pv.tile([128, L], dt.float32, tag="vps")
            for _ in range(14):
                nc.tensor.matmul(
                    wps[:],
                    warm_src[:, 0:256].rearrange("p (k m) -> p k m", k=2),
                    warm_src[:, 256:1280].rearrange("p (k n) -> p k n", k=2),
                    start=True, stop=True, perf_mode=DR,
                )

            # input DMAs (queue order = program order; first W columns first)
            for b in range(BPC):
                nc.sync.dma_start(
                    s1t8[:, b], s1T8[b].rearrange("(k p) i -> p k i", p=128))
            for b in range(BPC):
                nc.sync.dma_start(
                    s2t8[:, b], s2T8[b].rearrange("(k p) j -> p k j", p=128))
            nc.sync.dma_start(iot[:], iotac)

            # ---------------- Phase 1: fp8 DoubleRow ----------------
            for o in range(OUT_DIM):
                wt = wpool.tile([128, PCH, H], dt.float8e4, tag="wt")
                Wo = W8[o].rearrange("(k p) q -> p k q", p=128)
                nc.sync.dma_start(wt[:, :, 0:128], Wo[:, :, 0:128])
                nc.sync.dma_start(wt[:, :, 128:512], Wo[:, :, 128:512])
                nc.sync.dma_start(wt[:, :, 512:1024], Wo[:, :, 512:1024])
                if o == 0:
                    # rescore prefetches ride behind phase-1's first loads
                    for b in range(BPC):
                        nc.sync.dma_start(
                            s2tb[:, b], s2Tb[b].rearrange("(k p) j -> p k j", p=128))

                ats = [atp.tile([128, PCH, L], dt.float8e4, tag=f"at{b}",
                                name=f"at{b}") for b in range(BPC)]
                # stage 1: A.T chunks, W chunk stationary reused over 4 batches
                for qc in range(PCH):
                    accs = [pa.tile([128, L], dt.float32, tag=f"acc{b}",
                                    name=f"acc{b}") for b in range(BPC)]
                    for pp in range(QP):
                        for b in range(BPC):
                            nc.tensor.matmul(
                                accs[b][:],
                                wt[:, 2 * pp:2 * pp + 2, qc * 128:qc * 128 + 128],
                                s1t8[:, b, 2 * pp:2 * pp + 2, :],
                                start=(pp == 0), stop=(pp == QP - 1),
                                perf_mode=DR,
                            )
                    for b in range(BPC):
                        nc.scalar.mul(ats[b][:, qc, :], accs[b][:], A_SCALE)

                # stage 2: fp8 scores + per-row maxima + candidate row ids
                for b in range(BPC):
                    rm = rmp.tile([128, 4, 8], dt.float32, tag="rm")
                    for ic in range(4):
                        sc = ps.tile([128, L], dt.float32, tag="sc")
                        for qp in range(QP):
                            nc.tensor.matmul(
                                sc[:],
                                ats[b][:, 2 * qp:2 * qp + 2, ic * 128:ic * 128 + 128],
                                s2t8[:, b, 2 * qp:2 * qp + 2, :],
                                start=(qp == 0), stop=(qp == QP - 1),
                                perf_mode=DR,
                            )
                        nc.vector.max(rm[:, ic], sc[:])
                    # flatten row maxima: R[0, p*4+ic] = rm[p, ic, 0]
                    # (flat position j maps to score row (j%4)*128 + j//4;
                    #  host pre-permutes s1p rows to match)
                    R = rowp.tile([1, 512], dt.float32, tag="R")
                    nc.sync.dma_start(R[:], rm[:, :, 0])
                    Icat = smp.tile([1, NCAND], dt.uint32, tag="Icat")
                    Rcur = R
                    for rnd in range(NCAND // 8):
                        Tr = smp.tile([1, 8], dt.float32, tag=f"T{rnd}",
                                      name=f"T{rnd}")
                        nc.vector.max(Tr[:], Rcur[:])
                        nc.vector.max_index(Icat[:, 8 * rnd:8 * rnd + 8],
                                            Tr[:], Rcur[:])
                        if rnd < NCAND // 8 - 1:
                            Rn = rowp.tile([1, 512], dt.float32, tag=f"R{rnd}",
                                           name=f"R{rnd}")
                            nc.vector.match_replace(Rn[:], Tr[:], Rcur[:], -3.0e38)
                            Rcur = Rn
                    # stage the 32 ids (u32 -> exact f16) in partition-0 row
                    # of IfAll for the post-phase-1 PE broadcast
                    r = b * OUT_DIM + o
                    nc.vector.tensor_scalar_add(
                        IfAll[0:1, r * NCAND:(r + 1) * NCAND], Icat[:], 0.0)

            # ---------------- Phase 2: exact fp16 rescore ----------------
            # broadcast all 768 candidate ids to 128 partitions (2 PE matmuls)
            ibps = []
            for h, (pool, tg) in enumerate([(pv, "vps"), (ps, "sc")]):
                ibp = pool.tile([128, L], dt.float32, tag=tg, name=f"ibp{h}")
                nc.tensor.matmul(ibp[:, 0:NR * NCAND // 2], onesE[:],
                                 IfAll[:, h * (NR * NCAND // 2):(h + 1) * (NR * NCAND // 2)],
                                 start=True, stop=True)
                ibps.append(ibp)
            # one-hot gather matrices + PE gather of candidate s1 rows
            GT = gtp.tile([128, BPC, PCH, NCB], dt.float16, tag="gt")
            for b in range(BPC):
                gtb = gtp.tile([128, 4, NCB], dt.float16, tag=f"gtb{b}",
                               name=f"gtb{b}")
                for o in range(OUT_DIM):
                    r = b * OUT_DIM + o
                    ib = ibps[r // 12]
                    col = (r % 12) * NCAND
                    for jc in range(4):
                        nc.vector.tensor_scalar(
                            out=gtb[:, jc, o * NCAND:(o + 1) * NCAND],
                            in0=ib[:, col:col + NCAND],
                            scalar1=iot[:, jc:jc + 1], scalar2=None,
                            op0=mybir.AluOpType.is_equal)
                s1ps = gtp.tile([128, 4, H], dt.float16, tag="s1ps", bufs=2,
                                name="s1ps")
                nc.sync.dma_start(s1ps[:], s1p[b].rearrange("(k p) f -> p k f", p=128))
                for pc in range(PCH):
                    gps = pa.tile([128, L], dt.float32, tag=f"acc{b}",
                                  name="gps")
                    for jc in range(4):
                        nc.tensor.matmul(
                            gps[:, 0:NCB], s1ps[:, jc, pc * 128:pc * 128 + 128],
                            gtb[:, jc], start=(jc == 0), stop=(jc == 3))
                    nc.scalar.copy(GT[:, b, pc], gps[:, 0:NCB])

            for o in range(OUT_DIM):
                wb = wbp.tile([128, PCH, H], dt.float16, tag="wb")
                nc.sync.dma_start(wb[:], Wb[o].rearrange("(k p) q -> p k q", p=128))
                # V[q, (b,c)] = exact A rows = sum_p W[p,q] * s1[idx_c, p]
                vsb = vsp.tile([128, PCH, BPC, NCAND], dt.float16, tag="vsb")
                for qh in range(2):
                    vps = pv.tile([128, L], dt.float32, tag="vps", name="vps")
                    for q4 in range(4):
                        qc = 4 * qh + q4
                        for pc in range(PCH):
                            nc.tensor.matmul(
                                vps[:, q4 * 128:q4 * 128 + 128],
                                wb[:, pc, qc * 128:qc * 128 + 128],
                                GT[:, :, pc, o * NCAND:o * NCAND + NCAND],
                                start=(pc == 0), stop=(pc == PCH - 1),
                            )
                    nc.scalar.copy(
                        vsb[:, 4 * qh:4 * qh + 4].rearrange("p a b c -> p (a b c)"),
                        vps[:])
                # exact candidate scores, 4 batches col-packed in the PE
                fps = pa.tile([128, L], dt.float32, tag="fps")
                for qc in range(PCH):
                    for b in range(BPC):
                        nc.tensor.matmul(
                            fps[32 * b:32 * b + NCAND, :],
                            vsb[:, qc, b, :],
                            s2tb[:, b, qc, :],
                            start=(qc == 0), stop=(qc == PCH - 1),
                            tile_position=(0, 32 * b),
                            skip_group_check=True,
                        )
                for b in range(BPC):
                    nc.vector.max(cnd_all[32 * b:32 * b + NCAND, o * 8:o * 8 + 8],
                                  fps[32 * b:32 * b + NCAND, :])

            # final exact top-16 per (b, o) from 32 rows x top-8 candidates
            Fc = cndp.tile([NR, NCAND * 8], dt.float32, tag="fc")
            for b in range(BPC):
                for o in range(OUT_DIM):
                    r = b * OUT_DIM + o
                    nc.sync.dma_start(
                        Fc[r:r + 1, :].rearrange("a (g m) -> a g m", g=NCAND),
                        cnd_all[32 * b:32 * b + NCAND, o * 8:o * 8 + 8],
                    )
            t8 = cndp.tile([NR, 8], dt.float32, tag="t8")
            nc.vector.max(t8[:], Fc[:])
            nc.sync.dma_start(out[:, 0:8], t8[:])
            repl = cndp.tile([NR, NCAND * 8], dt.float32, tag="repl")
            nc.vector.match_replace(repl[:], t8[:], Fc[:], -3.0e38)
            n8 = cndp.tile([NR, 8], dt.float32, tag="n8")
            nc.vector.max(n8[:], repl[:])
            nc.sync.dma_start(out[:, 8:16], n8[:])

    nc.compile()
    return nc


# s1p row j holds s1 row (j%4)*128 + j//4, so selection-chain positions in
# the p-major rowmax flatten are directly row ids into s1p.
_PERM = np.array([(j % 4) * 128 + j // 4 for j in range(L)])
# iotac[p, jc] = jc*128 + p: the row id each (partition, chunk) of the
# gather matrices corresponds to.
_IOTAC = (np.arange(4)[None, :] * 128 + np.arange(128)[:, None]).astype(np.float32)


def _in_maps(sent1, sent2, W):
    f8 = dt.np(dt.float8e4)
    s1 = np.asarray(sent1)
    s2 = np.asarray(sent2)
    Wf = np.asarray(W)
    W8 = np.clip(Wf * W_SCALE, -240, 240).astype(f8)
    Wb = Wf.astype(np.float16)
    maps = []
    for c in range(NCORES):
        sl = slice(c * BPC, (c + 1) * BPC)
        s1c, s2c = s1[sl], s2[sl]
        maps.append({
            "s1T8": np.ascontiguousarray(s1c.transpose(0, 2, 1)).astype(f8),
            "s2T8": np.ascontiguousarray(s2c.transpose(0, 2, 1)).astype(f8),
            "W8": W8,
            "s1p": np.ascontiguousarray(s1c[:, _PERM, :]).astype(np.float16),
            "s2Tb": np.ascontiguousarray(s2c.transpose(0, 2, 1)).astype(np.float16),
            "Wb": Wb,
            "iotac": _IOTAC,
        })
    return maps


def _gather(results):
    outs = []
    for c in range(NCORES):
        o = results[c]["out"]                      # [24, 16]
        outs.append(o[:, :TOPK].reshape(BPC, OUT_DIM, TOPK))
    return np.concatenate(outs, axis=0).astype(np.float32)


def kernel(sent1, sent2, W):
    global _NC
    if _NC is None:
        _NC = _build()
    res = bass_utils.run_bass_kernel_spmd(
        _NC, _in_maps(sent1, sent2, W), core_ids=list(range(NCORES))
    )
    return _gather(res.results)


def run_traced(sent1, sent2, W):
    """Like kernel() but with NTFF tracing; returns (output, exec_time_ns, res).

    The caller must install the antenv.axon_hooks NTFF profile hook first
    (see test.py); without it exec_time_ns is None.
    """
    global _NC
    if _NC is None:
        _NC = _build()
    res = bass_utils.run_bass_kernel_spmd(
        _NC, _in_maps(sent1, sent2, W), core_ids=list(range(NCORES)), trace=True
    )
    return _gather(res.results), res.exec_time_ns, res
